# revision 30
# baseline (speedup 1.0000x reference)
"""Bahdanau-attention scoring kernel for Trainium2 (8 NeuronCores, data-parallel over batch).

Computes, for enc [S=2048, B=64, F=1024] f32 and hidden [B, 512] f32:
    energy    = tanh(cat([hidden_bcast, enc]) @ attn_w.T + attn_b)   # [S, B, 512]
    attention = energy @ v_w.T (+ v_b)                                # [S, B, 1]
    out       = softmax_over_S(attention / TEMP)                      # [S, B, 1]

v_b is a global scalar shift -> cancels in the softmax, dropped.

Transposed-stationary design: the PE matmuls put TOKENS on the output
partitions (stationary lhsT = 128-token fp8 block of x', moving rhs = the
attn weights). The energy PSUM comes out [128 tokens, 512 d], so:
  - the v-dot is a free-axis fused multiply+row-sum on the Vector engine
    (scalar_tensor_tensor with accum_out against a partition-replicated v;
    NB tensor_tensor_reduce wedges TRN2 hw) -- ZERO extra PE matmuls;
  - exp/softmax bookkeeping runs on 128 partitions (one small ACT exp per
    batch row with fused row-sum) instead of a single partition.
The per-partition ACT bias can't add h_proj (it varies along the free axis
here), so the hidden projection is folded into the DATA on the host:
    x' = enc[:, b, :] + W_dev^+ @ (hidden[b] @ W_h.T + attn_b)
an exact least-norm solve against the fp8-QUANTIZED device weights (W_dev
is full row rank; residual ~1e-14). This inflates the fp8 quantization
noise of x' by ~1.3x, still well inside tolerance, and the v-dot becomes
exact fp16/f32.

DMA: the whole 16.75 MB per-core x' stream is SBUF-RESIDENT (128KB of the
208KB usable per partition) and loaded by a few BIG per-batch-row DMAs
(16KB contiguous per partition) issued up-front and ungated. Dribbled
per-tile 4KB-row DMAs gated on compute only reach ~112 GB/s under 8-core
load; big early issue measures 310 GB/s/core -> the input stream takes
~55us and hides entirely under the ~110us PE stream. (b0/st0 is split
per-kp, interleaved with the per-kp weights on the second queue, so the
first matmul starts at ~10us; the bulk must stay split across both
hardware-DGE queues or ring dispatch collapses the rate.)

The softmax normalization (a per-b scalar sum + divide over the 2048
device-computed scores) happens in the host's gather/transpose pass: the
device ships exp(att/TEMP) [128, b, 16] as one consolidated big-row DMA
split across both queues' partition halves -- per-b 64B-row output DMAs
crawl ~300ns/packet and stall the end-of-kernel barrier by ~8us.

Weights are prescaled x32 against e4m3 subnormals (1/32 rides the tanh
activation scale).
"""
import os
import sys
import types

import numpy as np
import ml_dtypes

S = 2048
B = 64
F = 1024
D = 512
NCORES = 8
BLOC = B // NCORES  # 8
TEMP = 3.0
ST = 4          # s-tiles per batch row (S / 512)
TT = 512        # tokens per tile
Q = TT // 128   # 128-token blocks per tile
KP = F // 256   # 4 contraction chunks (256 features each, fp8 DoubleRow)
WSCALE = 32.0   # fp8 weight prescale (attn_w values are subnormal in e4m3 otherwise)


def _install_ntff_hook():
    """Make trace=True work under axon by registering the NTFF profile hook."""
    try:
        from antenv import axon_hooks  # noqa: F401
        return
    except ImportError:
        pass
    try:
        import antenv
        from trn_agent_boot.trn_boot import _ntff_profile_via_ctypes
        mod = types.ModuleType("antenv.axon_hooks")
        mod._hook = _ntff_profile_via_ctypes("/opt/axon/libaxon_pjrt.so")
        mod.set_axon_ntff_profile_hook = lambda h: setattr(mod, "_hook", h)
        mod.get_axon_ntff_profile_hook = lambda: mod._hook
        sys.modules["antenv.axon_hooks"] = mod
        antenv.axon_hooks = mod
    except Exception:
        pass


_NC_CACHE = {}


def _build():
    if "nc" in _NC_CACHE:
        return _NC_CACHE["nc"]
    import concourse.bacc as bacc
    import concourse.mybir as mybir
    from concourse.tile import TileContext

    f32 = mybir.dt.float32
    fp16 = mybir.dt.float16
    fp8 = mybir.dt.float8e4

    nc = bacc.Bacc("TRN2")
    # x' blocks: xin[b, p, st, kp, q, j, t]
    #   = x'[f = 256*kp + 2*p + j, s = st*512 + q*128 + t]  for batch row b.
    # Per-b slice is 16KB contiguous per partition -> big-packet DMA.
    xin = nc.dram_tensor("xin", [BLOC, 128, ST, KP, Q, 2, 128], fp8,
                         kind="ExternalInput")
    # moving weights: wt[p, kp, d, j] = fp8(32 * w_e[d, 256*kp + 2*p + j])
    wt = nc.dram_tensor("wt", [128, KP, D, 2], fp8, kind="ExternalInput")
    # v replicated across partitions
    vrep = nc.dram_tensor("vrep", [128, D], fp16, kind="ExternalInput")
    # output: per (token-part, b): 16 cols exp(att/TEMP). The softmax sum is
    # finished on the host (it already does the divide), so no on-device
    # row-sum/accum-read. One consolidated tensor -> one big-row DMA (per-b
    # 64B-row DMAs crawl at ~300ns/packet and stall the end barrier by ~8us).
    outd = nc.dram_tensor("out", [128, BLOC, ST * Q], f32, kind="ExternalOutput")

    tiles = [(b, st) for b in range(BLOC) for st in range(ST)]

    with TileContext(nc) as tc:
        with (
            tc.tile_pool(name="consts", bufs=1) as cpool,
            tc.tile_pool(name="work", bufs=1) as pool,
            tc.tile_pool(name="ps_e", bufs=8, space="PSUM") as pse,
        ):
            # whole x' stream resident in SBUF: 128KB/partition
            xt = cpool.tile([128, BLOC, ST, KP, Q, 2, 128], fp8)
            wt_sb = cpool.tile([128, KP, D, 2], fp8)
            vrep_sb = cpool.tile([128, D], fp16)

            # Up-front, ungated DMA issue. sync queue: the x' stream (b0/st0
            # split per-kp so compute starts early, the rest coarse).
            # scalar queue: weights (one 4KB-row DMA) + vrep.
            xc = xt.rearrange("p b st kp q j t -> p b st kp (q j t)")
            xv = xt.rearrange("p b st kp q j t -> p b st (kp q j t)")
            xin_c = xin[0].rearrange("p st kp q j t -> p st kp (q j t)")
            for kp in range(KP):
                nc.sync.dma_start(out=xc[:, 0, 0, kp], in_=xin_c[:, 0, kp])
                nc.scalar.dma_start(out=wt_sb[:, kp], in_=wt[:, kp])
            for st in range(1, ST):
                nc.sync.dma_start(
                    out=xv[:, 0, st],
                    in_=xin[0].rearrange("p st kp q j t -> p st (kp q j t)")[:, st],
                )
            nc.scalar.dma_start(out=vrep_sb[:], in_=vrep[:])
            xvb = xt.rearrange("p b st kp q j t -> p b (st kp q j t)")
            for b in range(1, BLOC):
                nc.sync.dma_start(
                    out=xvb[:, b],
                    in_=xin[b].rearrange("p st kp q j t -> p (st kp q j t)"),
                )


            out_sb = pool.tile([128, BLOC, ST * Q], f32, tag="osb", bufs=1,
                               name="osb")
            atts = {}

            def epilogue(b, st, q, ps):
                th = pool.tile([128, TT], fp16, tag="th", bufs=6, name="th")
                nc.scalar.activation(
                    th[:], ps[:], mybir.ActivationFunctionType.Tanh,
                    scale=float(1.0 / WSCALE),
                )
                # fused v-dot: out = th * vrep (scratch), accum = row-sum
                wscr = pool.tile([128, TT], fp16, tag="wscr", bufs=2, name="wscr")
                c = st * Q + q
                nc.vector.scalar_tensor_tensor(
                    out=wscr[:],
                    in0=th[:],
                    scalar=0.0,
                    in1=vrep_sb[:],
                    op0=mybir.AluOpType.bypass,
                    op1=mybir.AluOpType.mult,
                    accum_out=atts[b][:, c : c + 1],
                )

            for idx in range(len(tiles)):
                b, st = tiles[idx]
                if st == 0:
                    atts[b] = pool.tile([128, ST * Q], f32, tag="atts", bufs=2,
                                        name=f"at{b}")
                if idx == 0:
                    # first tile kp-major: each arriving kp chunk unlocks 4
                    # matmuls (852ns of PE work) instead of 1, hiding the
                    # serial arrival of the startup chunks
                    pss = [pse.tile([128, TT], f32, tag="ps", name="ps")
                           for _ in range(Q)]
                    for kp in range(KP):
                        for q in range(Q):
                            nc.tensor.matmul(
                                pss[q][:],
                                lhsT=xt[:, b, st, kp, q],
                                rhs=wt_sb[:, kp].rearrange("p d j -> p j d"),
                                start=(kp == 0),
                                stop=(kp == KP - 1),
                                perf_mode=mybir.MatmulPerfMode.DoubleRow,
                            )
                    for q in range(Q):
                        epilogue(b, st, q, pss[q])
                else:
                    for q in range(Q):
                        ps = pse.tile([128, TT], f32, tag="ps", name="ps")
                        for kp in range(KP):
                            nc.tensor.matmul(
                                ps[:],
                                lhsT=xt[:, b, st, kp, q],
                                rhs=wt_sb[:, kp].rearrange("p d j -> p j d"),
                                start=(kp == 0),
                                stop=(kp == KP - 1),
                                perf_mode=mybir.MatmulPerfMode.DoubleRow,
                            )
                        epilogue(b, st, q, ps)
                if st == ST - 1:
                    nc.scalar.activation(
                        out_sb[:, b], atts[b][:],
                        mybir.ActivationFunctionType.Exp,
                        scale=float(1.0 / TEMP),
                    )
            # one consolidated output DMA, split across both queues'
            # partition halves so the two resolve in parallel
            nc.sync.dma_start(
                out=outd[:64].rearrange("p b c -> p (b c)"),
                in_=out_sb[:64].rearrange("p b c -> p (b c)"),
            )
            nc.scalar.dma_start(
                out=outd[64:].rearrange("p b c -> p (b c)"),
                in_=out_sb[64:].rearrange("p b c -> p (b c)"),
            )

    nc.compile()
    _NC_CACHE["nc"] = nc
    return nc


def _prep(hidden, encoder_outputs, attn_w, attn_b, v_w):
    """Host prep: fold h_proj into x via least-norm solve vs quantized weights,
    quantize to fp8, and transpose to the per-tile stationary layout."""
    fp8np = ml_dtypes.float8_e4m3

    w_e = attn_w[:, D:]                               # [D, F]
    wt8 = (w_e * WSCALE).astype(fp8np)                # device weights
    w_dev = wt8.astype(np.float64) / WSCALE

    h_proj = hidden.astype(np.float64) @ attn_w[:, :D].T.astype(np.float64) + attn_b
    pinv = np.linalg.pinv(w_dev)                      # [F, D]
    dx = (pinv @ h_proj.T).T.astype(np.float32)       # [B, F]

    xq = (encoder_outputs + dx[None, :, :]).astype(fp8np)   # [S, B, F]
    # [S,B,F] -> [st, q, t, B, kp, p, j] -> [B, p, st, kp, q, j, t]
    v = xq.reshape(ST, Q, 128, B, KP, 128, 2).transpose(3, 5, 0, 4, 1, 6, 2)
    xin = np.ascontiguousarray(v)                     # [B, 128, ST, KP, Q, 2, 128]

    # wt[p, kp, d, j] = wt8[d, 256*kp + 2*p + j]
    wtl = np.ascontiguousarray(wt8.reshape(D, KP, 128, 2).transpose(2, 1, 0, 3))

    vrep = np.ascontiguousarray(
        np.broadcast_to(v_w[0].astype(np.float16)[None, :], (128, D))
    )
    return xin, wtl, vrep


def kernel(hidden, encoder_outputs, attn_w, attn_b, v_w, v_b):
    _install_ntff_hook()
    from concourse.bass_utils import run_bass_kernel_spmd

    hidden = np.asarray(hidden, dtype=np.float32)
    encoder_outputs = np.asarray(encoder_outputs, dtype=np.float32)
    attn_w = np.asarray(attn_w, dtype=np.float32)
    attn_b = np.asarray(attn_b, dtype=np.float32)
    v_w = np.asarray(v_w, dtype=np.float32)

    nc = _build()
    xin_full, wtl, vrep = _prep(hidden, encoder_outputs, attn_w, attn_b, v_w)

    in_maps = []
    for c in range(NCORES):
        b0 = c * BLOC
        in_maps.append(
            {
                "xin": np.ascontiguousarray(xin_full[b0 : b0 + BLOC]),
                "wt": wtl,
                "vrep": vrep,
            }
        )

    trace = bool(int(os.environ.get("KERNEL_TRACE", "0")))
    res = run_bass_kernel_spmd(
        nc, in_maps, core_ids=list(range(NCORES)), trace=trace
    )
    kernel.last_result = res

    outs = []
    for c in range(NCORES):
        ex = res.results[c]["out"]         # [128, BLOC, 16]
        sums = ex.astype(np.float64).sum(axis=(0, 2))           # [BLOC]
        o = ex / sums[None, :, None].astype(np.float32)
        # o[t, b, st*4+q] -> [b, s = st*512 + q*128 + t]
        o = o.reshape(128, BLOC, ST, Q).transpose(1, 2, 3, 0).reshape(BLOC, S)
        outs.append(o)
    full = np.concatenate(outs, axis=0)    # [B, S]
    full = full.transpose(1, 0).reshape(S, B, 1)
    return np.ascontiguousarray(full, dtype=np.float32)


kernel.last_result = None


# revision 31
# speedup vs baseline: 1.1876x; 1.1876x over previous
"""Bahdanau-attention scoring kernel for Trainium2 (8 NeuronCores, data-parallel over batch).

Computes, for enc [S=2048, B=64, F=1024] f32 and hidden [B, 512] f32:
    energy    = tanh(cat([hidden_bcast, enc]) @ attn_w.T + attn_b)   # [S, B, 512]
    attention = energy @ v_w.T (+ v_b)                                # [S, B, 1]
    out       = softmax_over_S(attention / TEMP)                      # [S, B, 1]

v_b is a global scalar shift -> cancels in the softmax, dropped.

Transposed-stationary design: the PE matmuls put TOKENS on the output
partitions (stationary lhsT = 128-token fp8 block of x', moving rhs = the
attn weights). The energy PSUM comes out [128 tokens, 512 d], so:
  - the v-dot is a free-axis fused multiply+row-sum on the Vector engine
    (scalar_tensor_tensor with accum_out against a partition-replicated v;
    NB tensor_tensor_reduce wedges TRN2 hw) -- ZERO extra PE matmuls;
  - exp/softmax bookkeeping runs on 128 partitions (one small ACT exp per
    batch row with fused row-sum) instead of a single partition.
The per-partition ACT bias can't add h_proj (it varies along the free axis
here), so the hidden projection is folded into the DATA on the host:
    x' = enc[:, b, :] + W_dev^+ @ (hidden[b] @ W_h.T + attn_b)
an exact least-norm solve against the fp8-QUANTIZED device weights (W_dev
is full row rank; residual ~1e-14). This inflates the fp8 quantization
noise of x' by ~1.3x, still well inside tolerance, and the v-dot becomes
exact fp16/f32.

DMA: the whole 16.75 MB per-core x' stream is SBUF-RESIDENT (128KB of the
208KB usable per partition) and loaded by a few BIG per-batch-row DMAs
(16KB contiguous per partition) issued up-front and ungated. Dribbled
per-tile 4KB-row DMAs gated on compute only reach ~112 GB/s under 8-core
load; big early issue measures 310 GB/s/core -> the input stream takes
~55us and hides entirely under the ~110us PE stream. (b0/st0 is split
per-kp, interleaved with the per-kp weights on the second queue, so the
first matmul starts at ~10us; the bulk must stay split across both
hardware-DGE queues or ring dispatch collapses the rate.)

The softmax normalization (a per-b scalar sum + divide over the 2048
device-computed scores) happens in the host's gather/transpose pass: the
device ships exp(att/TEMP) [128, b, 16] as one consolidated big-row DMA
split across both queues' partition halves -- per-b 64B-row output DMAs
crawl ~300ns/packet and stall the end-of-kernel barrier by ~8us.

Weights are prescaled x32 against e4m3 subnormals (1/32 rides the tanh
activation scale).
"""
import os
import sys
import types

import numpy as np
import ml_dtypes

S = 2048
B = 64
F = 1024
D = 512
NCORES = 8
BLOC = B // NCORES  # 8
TEMP = 3.0
ST = 4          # s-tiles per batch row (S / 512)
TT = 512        # tokens per tile
Q = TT // 128   # 128-token blocks per tile
KP = F // 256   # 4 contraction chunks (256 features each, fp8 DoubleRow)
WSCALE = 32.0   # fp8 weight prescale (attn_w values are subnormal in e4m3 otherwise)


def _install_ntff_hook():
    """Make trace=True work under axon by registering the NTFF profile hook."""
    try:
        from antenv import axon_hooks  # noqa: F401
        return
    except ImportError:
        pass
    try:
        import antenv
        from trn_agent_boot.trn_boot import _ntff_profile_via_ctypes
        mod = types.ModuleType("antenv.axon_hooks")
        mod._hook = _ntff_profile_via_ctypes("/opt/axon/libaxon_pjrt.so")
        mod.set_axon_ntff_profile_hook = lambda h: setattr(mod, "_hook", h)
        mod.get_axon_ntff_profile_hook = lambda: mod._hook
        sys.modules["antenv.axon_hooks"] = mod
        antenv.axon_hooks = mod
    except Exception:
        pass


_NC_CACHE = {}


def _build():
    if "nc" in _NC_CACHE:
        return _NC_CACHE["nc"]
    import concourse.bacc as bacc
    import concourse.mybir as mybir
    from concourse.tile import TileContext

    f32 = mybir.dt.float32
    fp16 = mybir.dt.float16
    fp8 = mybir.dt.float8e4

    nc = bacc.Bacc("TRN2")
    # x' blocks: xin[b, p, st, kp, q, j, t]
    #   = x'[f = 256*kp + 2*p + j, s = st*512 + q*128 + t]  for batch row b.
    # Per-b slice is 16KB contiguous per partition -> big-packet DMA.
    xin = nc.dram_tensor("xin", [BLOC, 128, ST, KP, Q, 2, 128], fp8,
                         kind="ExternalInput")
    # moving weights: wt[p, kp, d, j] = fp8(32 * w_e[d, 256*kp + 2*p + j])
    wt = nc.dram_tensor("wt", [128, KP, D, 2], fp8, kind="ExternalInput")
    # v replicated across partitions
    vrep = nc.dram_tensor("vrep", [128, D], fp16, kind="ExternalInput")
    # output: per (token-part, b): 16 cols exp(att/TEMP). The softmax sum is
    # finished on the host (it already does the divide), so no on-device
    # row-sum/accum-read. One consolidated tensor -> one big-row DMA (per-b
    # 64B-row DMAs crawl at ~300ns/packet and stall the end barrier by ~8us).
    outd = nc.dram_tensor("out", [128, BLOC, ST * Q], f32, kind="ExternalOutput")

    tiles = [(b, st) for b in range(BLOC) for st in range(ST)]

    with TileContext(nc) as tc:
        with (
            tc.tile_pool(name="consts", bufs=1) as cpool,
            tc.tile_pool(name="work", bufs=1) as pool,
            tc.tile_pool(name="ps_e", bufs=8, space="PSUM") as pse,
        ):
            # whole x' stream resident in SBUF: 128KB/partition
            xt = cpool.tile([128, BLOC, ST, KP, Q, 2, 128], fp8)
            wt_sb = cpool.tile([128, KP, D, 2], fp8)
            vrep_sb = cpool.tile([128, D], fp16)

            # Up-front, ungated DMA issue. sync queue: the x' stream (b0/st0
            # split per-kp so compute starts early, the rest coarse).
            # scalar queue: weights (one 4KB-row DMA) + vrep.
            xc = xt.rearrange("p b st kp q j t -> p b st kp (q j t)")
            xv = xt.rearrange("p b st kp q j t -> p b st (kp q j t)")
            xin_c = xin[0].rearrange("p st kp q j t -> p st kp (q j t)")
            for kp in range(KP):
                nc.sync.dma_start(out=xc[:, 0, 0, kp], in_=xin_c[:, 0, kp])
                nc.scalar.dma_start(out=wt_sb[:, kp], in_=wt[:, kp])
            for st in range(1, ST):
                nc.sync.dma_start(
                    out=xv[:, 0, st],
                    in_=xin[0].rearrange("p st kp q j t -> p st (kp q j t)")[:, st],
                )
            nc.scalar.dma_start(out=vrep_sb[:], in_=vrep[:])
            xvb = xt.rearrange("p b st kp q j t -> p b (st kp q j t)")
            for b in range(1, BLOC):
                nc.sync.dma_start(
                    out=xvb[:, b],
                    in_=xin[b].rearrange("p st kp q j t -> p (st kp q j t)"),
                )


            # Bare-LDWEIGHTS block spanning the initial DMA wait (array-busy
            # 7.6-10.4us). Measured neutral-to-slightly-positive; kept because
            # the best-measured config included it.
            warm = pool.tile([128, 128], fp8, tag="warm", bufs=1, name="warm")
            nc.vector.memset(warm[:], 0.25)
            for _ in range(26):
                nc.tensor.ldweights(warm[:])

            out_sb = pool.tile([128, BLOC, ST * Q], f32, tag="osb", bufs=1,
                               name="osb")
            atts = {}

            def epilogue(b, st, q, ps):
                th = pool.tile([128, TT], fp16, tag="th", bufs=6, name="th")
                nc.scalar.activation(
                    th[:], ps[:], mybir.ActivationFunctionType.Tanh,
                    scale=float(1.0 / WSCALE),
                )
                # fused v-dot: out = th * vrep (scratch), accum = row-sum
                wscr = pool.tile([128, TT], fp16, tag="wscr", bufs=2, name="wscr")
                c = st * Q + q
                nc.vector.scalar_tensor_tensor(
                    out=wscr[:],
                    in0=th[:],
                    scalar=0.0,
                    in1=vrep_sb[:],
                    op0=mybir.AluOpType.bypass,
                    op1=mybir.AluOpType.mult,
                    accum_out=atts[b][:, c : c + 1],
                )

            for idx in range(len(tiles)):
                b, st = tiles[idx]
                if st == 0:
                    atts[b] = pool.tile([128, ST * Q], f32, tag="atts", bufs=2,
                                        name=f"at{b}")
                if idx == 0:
                    # first tile kp-major: each arriving kp chunk unlocks 4
                    # matmuls (852ns of PE work) instead of 1, hiding the
                    # serial arrival of the startup chunks
                    pss = [pse.tile([128, TT], f32, tag="ps", name="ps")
                           for _ in range(Q)]
                    for kp in range(KP):
                        for q in range(Q):
                            nc.tensor.matmul(
                                pss[q][:],
                                lhsT=xt[:, b, st, kp, q],
                                rhs=wt_sb[:, kp].rearrange("p d j -> p j d"),
                                start=(kp == 0),
                                stop=(kp == KP - 1),
                                perf_mode=mybir.MatmulPerfMode.DoubleRow,
                            )
                    for q in range(Q):
                        epilogue(b, st, q, pss[q])
                else:
                    for q in range(Q):
                        ps = pse.tile([128, TT], f32, tag="ps", name="ps")
                        for kp in range(KP):
                            nc.tensor.matmul(
                                ps[:],
                                lhsT=xt[:, b, st, kp, q],
                                rhs=wt_sb[:, kp].rearrange("p d j -> p j d"),
                                start=(kp == 0),
                                stop=(kp == KP - 1),
                                perf_mode=mybir.MatmulPerfMode.DoubleRow,
                            )
                        epilogue(b, st, q, ps)
                if st == ST - 1:
                    nc.scalar.activation(
                        out_sb[:, b], atts[b][:],
                        mybir.ActivationFunctionType.Exp,
                        scale=float(1.0 / TEMP),
                    )
            # one consolidated output DMA, split across both queues'
            # partition halves so the two resolve in parallel
            nc.sync.dma_start(
                out=outd[:64].rearrange("p b c -> p (b c)"),
                in_=out_sb[:64].rearrange("p b c -> p (b c)"),
            )
            nc.scalar.dma_start(
                out=outd[64:].rearrange("p b c -> p (b c)"),
                in_=out_sb[64:].rearrange("p b c -> p (b c)"),
            )

    nc.compile()
    _NC_CACHE["nc"] = nc
    return nc


def _prep(hidden, encoder_outputs, attn_w, attn_b, v_w):
    """Host prep: fold h_proj into x via least-norm solve vs quantized weights,
    quantize to fp8, and transpose to the per-tile stationary layout."""
    fp8np = ml_dtypes.float8_e4m3

    w_e = attn_w[:, D:]                               # [D, F]
    wt8 = (w_e * WSCALE).astype(fp8np)                # device weights
    w_dev = wt8.astype(np.float64) / WSCALE

    h_proj = hidden.astype(np.float64) @ attn_w[:, :D].T.astype(np.float64) + attn_b
    pinv = np.linalg.pinv(w_dev)                      # [F, D]
    dx = (pinv @ h_proj.T).T.astype(np.float32)       # [B, F]

    xq = (encoder_outputs + dx[None, :, :]).astype(fp8np)   # [S, B, F]
    # [S,B,F] -> [st, q, t, B, kp, p, j] -> [B, p, st, kp, q, j, t]
    v = xq.reshape(ST, Q, 128, B, KP, 128, 2).transpose(3, 5, 0, 4, 1, 6, 2)
    xin = np.ascontiguousarray(v)                     # [B, 128, ST, KP, Q, 2, 128]

    # wt[p, kp, d, j] = wt8[d, 256*kp + 2*p + j]
    wtl = np.ascontiguousarray(wt8.reshape(D, KP, 128, 2).transpose(2, 1, 0, 3))

    vrep = np.ascontiguousarray(
        np.broadcast_to(v_w[0].astype(np.float16)[None, :], (128, D))
    )
    return xin, wtl, vrep


def kernel(hidden, encoder_outputs, attn_w, attn_b, v_w, v_b):
    _install_ntff_hook()
    from concourse.bass_utils import run_bass_kernel_spmd

    hidden = np.asarray(hidden, dtype=np.float32)
    encoder_outputs = np.asarray(encoder_outputs, dtype=np.float32)
    attn_w = np.asarray(attn_w, dtype=np.float32)
    attn_b = np.asarray(attn_b, dtype=np.float32)
    v_w = np.asarray(v_w, dtype=np.float32)

    nc = _build()
    xin_full, wtl, vrep = _prep(hidden, encoder_outputs, attn_w, attn_b, v_w)

    in_maps = []
    for c in range(NCORES):
        b0 = c * BLOC
        in_maps.append(
            {
                "xin": np.ascontiguousarray(xin_full[b0 : b0 + BLOC]),
                "wt": wtl,
                "vrep": vrep,
            }
        )

    trace = bool(int(os.environ.get("KERNEL_TRACE", "0")))
    res = run_bass_kernel_spmd(
        nc, in_maps, core_ids=list(range(NCORES)), trace=trace
    )
    kernel.last_result = res

    outs = []
    for c in range(NCORES):
        ex = res.results[c]["out"]         # [128, BLOC, 16]
        sums = ex.astype(np.float64).sum(axis=(0, 2))           # [BLOC]
        o = ex / sums[None, :, None].astype(np.float32)
        # o[t, b, st*4+q] -> [b, s = st*512 + q*128 + t]
        o = o.reshape(128, BLOC, ST, Q).transpose(1, 2, 3, 0).reshape(BLOC, S)
        outs.append(o)
    full = np.concatenate(outs, axis=0)    # [B, S]
    full = full.transpose(1, 0).reshape(S, B, 1)
    return np.ascontiguousarray(full, dtype=np.float32)


kernel.last_result = None


# revision 32
# speedup vs baseline: 1.1938x; 1.0052x over previous
"""Bahdanau-attention scoring kernel for Trainium2 (8 NeuronCores, data-parallel over batch).

Computes, for enc [S=2048, B=64, F=1024] f32 and hidden [B, 512] f32:
    energy    = tanh(cat([hidden_bcast, enc]) @ attn_w.T + attn_b)   # [S, B, 512]
    attention = energy @ v_w.T (+ v_b)                                # [S, B, 1]
    out       = softmax_over_S(attention / TEMP)                      # [S, B, 1]

v_b is a global scalar shift -> cancels in the softmax, dropped.

Transposed-stationary design: the PE matmuls put TOKENS on the output
partitions (stationary lhsT = 128-token fp8 block of x', moving rhs = the
attn weights). The energy PSUM comes out [128 tokens, 512 d], so:
  - the v-dot is a free-axis fused multiply+row-sum on the Vector engine
    (scalar_tensor_tensor with accum_out against a partition-replicated v;
    NB tensor_tensor_reduce wedges TRN2 hw) -- ZERO extra PE matmuls;
  - exp/softmax bookkeeping runs on 128 partitions (one small ACT exp per
    batch row with fused row-sum) instead of a single partition.
The per-partition ACT bias can't add h_proj (it varies along the free axis
here), so the hidden projection is folded into the DATA on the host:
    x' = enc[:, b, :] + W_dev^+ @ (hidden[b] @ W_h.T + attn_b)
an exact least-norm solve against the fp8-QUANTIZED device weights (W_dev
is full row rank; residual ~1e-14). This inflates the fp8 quantization
noise of x' by ~1.3x, still well inside tolerance, and the v-dot becomes
exact fp16/f32.

DMA: the whole 16.75 MB per-core x' stream is SBUF-RESIDENT (128KB of the
208KB usable per partition) and loaded by a few BIG per-batch-row DMAs
(16KB contiguous per partition) issued up-front and ungated. Dribbled
per-tile 4KB-row DMAs gated on compute only reach ~112 GB/s under 8-core
load; big early issue measures 310 GB/s/core -> the input stream takes
~55us and hides entirely under the ~110us PE stream. (b0/st0 is split
per-kp, interleaved with the per-kp weights on the second queue, so the
first matmul starts at ~10us; the bulk must stay split across both
hardware-DGE queues or ring dispatch collapses the rate.)

The softmax normalization (a per-b scalar sum + divide over the 2048
device-computed scores) happens in the host's gather/transpose pass: the
device ships exp(att/TEMP) [128, b, 16] as one consolidated big-row DMA
split across both queues' partition halves -- per-b 64B-row output DMAs
crawl ~300ns/packet and stall the end-of-kernel barrier by ~8us.

Weights are prescaled x32 against e4m3 subnormals (1/32 rides the tanh
activation scale).
"""
import os
import sys
import types

import numpy as np
import ml_dtypes

S = 2048
B = 64
F = 1024
D = 512
NCORES = 8
BLOC = B // NCORES  # 8
TEMP = 3.0
ST = 4          # s-tiles per batch row (S / 512)
TT = 512        # tokens per tile
Q = TT // 128   # 128-token blocks per tile
KP = F // 256   # 4 contraction chunks (256 features each, fp8 DoubleRow)
WSCALE = 32.0   # fp8 weight prescale (attn_w values are subnormal in e4m3 otherwise)


def _install_ntff_hook():
    """Make trace=True work under axon by registering the NTFF profile hook."""
    try:
        from antenv import axon_hooks  # noqa: F401
        return
    except ImportError:
        pass
    try:
        import antenv
        from trn_agent_boot.trn_boot import _ntff_profile_via_ctypes
        mod = types.ModuleType("antenv.axon_hooks")
        mod._hook = _ntff_profile_via_ctypes("/opt/axon/libaxon_pjrt.so")
        mod.set_axon_ntff_profile_hook = lambda h: setattr(mod, "_hook", h)
        mod.get_axon_ntff_profile_hook = lambda: mod._hook
        sys.modules["antenv.axon_hooks"] = mod
        antenv.axon_hooks = mod
    except Exception:
        pass


_NC_CACHE = {}


def _build():
    if "nc" in _NC_CACHE:
        return _NC_CACHE["nc"]
    import concourse.bacc as bacc
    import concourse.mybir as mybir
    from concourse.tile import TileContext

    f32 = mybir.dt.float32
    fp16 = mybir.dt.float16
    fp8 = mybir.dt.float8e4

    nc = bacc.Bacc("TRN2")
    # x' blocks: xin[b, p, st, kp, q, j, t]
    #   = x'[f = 256*kp + 2*p + j, s = st*512 + q*128 + t]  for batch row b.
    # Per-b slice is 16KB contiguous per partition -> big-packet DMA.
    xin = nc.dram_tensor("xin", [BLOC, 128, ST, KP, Q, 2, 128], fp8,
                         kind="ExternalInput")
    # moving weights: wt[p, kp, d, j] = fp8(32 * w_e[d, 256*kp + 2*p + j])
    wt = nc.dram_tensor("wt", [128, KP, D, 2], fp8, kind="ExternalInput")
    # v replicated across partitions
    vrep = nc.dram_tensor("vrep", [128, D], fp16, kind="ExternalInput")
    # output: per (token-part, b): 16 cols exp(att/TEMP). The softmax sum is
    # finished on the host (it already does the divide), so no on-device
    # row-sum/accum-read. One consolidated tensor -> one big-row DMA (per-b
    # 64B-row DMAs crawl at ~300ns/packet and stall the end barrier by ~8us).
    outd = nc.dram_tensor("out", [128, BLOC, ST * Q], f32, kind="ExternalOutput")

    tiles = [(b, st) for b in range(BLOC) for st in range(ST)]

    with TileContext(nc) as tc:
        with (
            tc.tile_pool(name="consts", bufs=1) as cpool,
            tc.tile_pool(name="work", bufs=1) as pool,
            tc.tile_pool(name="ps_e", bufs=8, space="PSUM") as pse,
        ):
            # whole x' stream resident in SBUF: 128KB/partition
            xt = cpool.tile([128, BLOC, ST, KP, Q, 2, 128], fp8)
            wt_sb = cpool.tile([128, KP, D, 2], fp8)
            vrep_sb = cpool.tile([128, D], fp16)

            # Up-front, ungated DMA issue. sync queue: the x' stream (b0/st0
            # split per-kp so compute starts early, the rest coarse).
            # scalar queue: weights (one 4KB-row DMA) + vrep.
            xc = xt.rearrange("p b st kp q j t -> p b st kp (q j t)")
            xv = xt.rearrange("p b st kp q j t -> p b st (kp q j t)")
            xin_c = xin[0].rearrange("p st kp q j t -> p st kp (q j t)")
            for kp in range(KP):
                nc.sync.dma_start(out=xc[:, 0, 0, kp], in_=xin_c[:, 0, kp])
                nc.scalar.dma_start(out=wt_sb[:, kp], in_=wt[:, kp])
            for st in range(1, ST):
                nc.sync.dma_start(
                    out=xv[:, 0, st],
                    in_=xin[0].rearrange("p st kp q j t -> p st (kp q j t)")[:, st],
                )
            nc.scalar.dma_start(out=vrep_sb[:], in_=vrep[:])
            xvb = xt.rearrange("p b st kp q j t -> p b (st kp q j t)")
            for b in range(1, BLOC):
                nc.sync.dma_start(
                    out=xvb[:, b],
                    in_=xin[b].rearrange("p st kp q j t -> p (st kp q j t)"),
                )


            # Bare-LDWEIGHTS block spanning the initial DMA wait (array-busy
            # 7.6-10.4us). Measured neutral-to-slightly-positive; kept because
            # the best-measured config included it.
            warm = pool.tile([128, 128], fp8, tag="warm", bufs=1, name="warm")
            nc.vector.memset(warm[:], 0.25)
            for _ in range(26):
                nc.tensor.ldweights(warm[:])

            out_sb = pool.tile([128, BLOC, ST * Q], f32, tag="osb", bufs=1,
                               name="osb")
            atts = {}

            def epilogue(b, st, q, ps):
                th = pool.tile([128, TT], fp16, tag="th", bufs=6, name="th")
                nc.scalar.activation(
                    th[:], ps[:], mybir.ActivationFunctionType.Tanh,
                    scale=float(1.0 / WSCALE),
                )
                # fused v-dot: out = th * vrep (scratch), accum = row-sum
                wscr = pool.tile([128, TT], fp16, tag="wscr", bufs=2, name="wscr")
                c = st * Q + q
                nc.vector.scalar_tensor_tensor(
                    out=wscr[:],
                    in0=th[:],
                    scalar=0.0,
                    in1=vrep_sb[:],
                    op0=mybir.AluOpType.bypass,
                    op1=mybir.AluOpType.mult,
                    accum_out=atts[b][:, c : c + 1],
                )

            for idx in range(len(tiles)):
                b, st = tiles[idx]
                if st == 0:
                    atts[b] = pool.tile([128, ST * Q], f32, tag="atts", bufs=2,
                                        name=f"at{b}")
                if idx == 0:
                    # first tile kp-major: each arriving kp chunk unlocks 4
                    # matmuls (852ns of PE work) instead of 1, hiding the
                    # serial arrival of the startup chunks
                    pss = [pse.tile([128, TT], f32, tag="ps", name="ps")
                           for _ in range(Q)]
                    for kp in range(KP):
                        for q in range(Q):
                            nc.tensor.matmul(
                                pss[q][:],
                                lhsT=xt[:, b, st, kp, q],
                                rhs=wt_sb[:, kp].rearrange("p d j -> p j d"),
                                start=(kp == 0),
                                stop=(kp == KP - 1),
                                perf_mode=mybir.MatmulPerfMode.DoubleRow,
                            )
                    for q in range(Q):
                        epilogue(b, st, q, pss[q])
                elif idx == len(tiles) - 1:
                    # last tile: split the final q-block along d into two
                    # N=256 halves (all-kp lo, then all-kp hi) so the lo
                    # tanh/v-dot overlaps the hi matmuls -- shortens the
                    # end-of-kernel chain by ~0.5us
                    for q in range(Q - 1):
                        ps = pse.tile([128, TT], f32, tag="ps", name="ps")
                        for kp in range(KP):
                            nc.tensor.matmul(
                                ps[:],
                                lhsT=xt[:, b, st, kp, q],
                                rhs=wt_sb[:, kp].rearrange("p d j -> p j d"),
                                start=(kp == 0),
                                stop=(kp == KP - 1),
                                perf_mode=mybir.MatmulPerfMode.DoubleRow,
                            )
                        epilogue(b, st, q, ps)
                    q = Q - 1
                    acc_h = pool.tile([128, 2], f32, tag="acch", bufs=1,
                                      name="acch")
                    for h in range(2):
                        psh = pse.tile([128, TT // 2], f32, tag="ps", name="ps")
                        dsl = slice(h * (D // 2), (h + 1) * (D // 2))
                        for kp in range(KP):
                            nc.tensor.matmul(
                                psh[:],
                                lhsT=xt[:, b, st, kp, q],
                                rhs=wt_sb[:, kp].rearrange("p d j -> p j d")[:, :, dsl],
                                start=(kp == 0),
                                stop=(kp == KP - 1),
                                perf_mode=mybir.MatmulPerfMode.DoubleRow,
                            )
                        thh = pool.tile([128, TT // 2], fp16, tag="th", bufs=6,
                                        name="th")
                        nc.scalar.activation(
                            thh[:], psh[:], mybir.ActivationFunctionType.Tanh,
                            scale=float(1.0 / WSCALE),
                        )
                        wsh = pool.tile([128, TT // 2], fp16, tag="wscr",
                                        bufs=2, name="wscr")
                        nc.vector.scalar_tensor_tensor(
                            out=wsh[:],
                            in0=thh[:],
                            scalar=0.0,
                            in1=vrep_sb[:, dsl],
                            op0=mybir.AluOpType.bypass,
                            op1=mybir.AluOpType.mult,
                            accum_out=acc_h[:, h : h + 1],
                        )
                    nc.vector.tensor_tensor(
                        out=atts[b][:, st * Q + q : st * Q + q + 1],
                        in0=acc_h[:, 0:1],
                        in1=acc_h[:, 1:2],
                        op=mybir.AluOpType.add,
                    )
                else:
                    for q in range(Q):
                        ps = pse.tile([128, TT], f32, tag="ps", name="ps")
                        for kp in range(KP):
                            nc.tensor.matmul(
                                ps[:],
                                lhsT=xt[:, b, st, kp, q],
                                rhs=wt_sb[:, kp].rearrange("p d j -> p j d"),
                                start=(kp == 0),
                                stop=(kp == KP - 1),
                                perf_mode=mybir.MatmulPerfMode.DoubleRow,
                            )
                        epilogue(b, st, q, ps)
                if st == ST - 1:
                    nc.scalar.activation(
                        out_sb[:, b], atts[b][:],
                        mybir.ActivationFunctionType.Exp,
                        scale=float(1.0 / TEMP),
                    )
            # one consolidated output DMA, split across both queues'
            # partition halves so the two resolve in parallel
            nc.sync.dma_start(
                out=outd[:64].rearrange("p b c -> p (b c)"),
                in_=out_sb[:64].rearrange("p b c -> p (b c)"),
            )
            nc.scalar.dma_start(
                out=outd[64:].rearrange("p b c -> p (b c)"),
                in_=out_sb[64:].rearrange("p b c -> p (b c)"),
            )

    nc.compile()
    _NC_CACHE["nc"] = nc
    return nc


def _prep(hidden, encoder_outputs, attn_w, attn_b, v_w):
    """Host prep: fold h_proj into x via least-norm solve vs quantized weights,
    quantize to fp8, and transpose to the per-tile stationary layout."""
    fp8np = ml_dtypes.float8_e4m3

    w_e = attn_w[:, D:]                               # [D, F]
    wt8 = (w_e * WSCALE).astype(fp8np)                # device weights
    w_dev = wt8.astype(np.float64) / WSCALE

    h_proj = hidden.astype(np.float64) @ attn_w[:, :D].T.astype(np.float64) + attn_b
    pinv = np.linalg.pinv(w_dev)                      # [F, D]
    dx = (pinv @ h_proj.T).T.astype(np.float32)       # [B, F]

    xq = (encoder_outputs + dx[None, :, :]).astype(fp8np)   # [S, B, F]
    # [S,B,F] -> [st, q, t, B, kp, p, j] -> [B, p, st, kp, q, j, t]
    v = xq.reshape(ST, Q, 128, B, KP, 128, 2).transpose(3, 5, 0, 4, 1, 6, 2)
    xin = np.ascontiguousarray(v)                     # [B, 128, ST, KP, Q, 2, 128]

    # wt[p, kp, d, j] = wt8[d, 256*kp + 2*p + j]
    wtl = np.ascontiguousarray(wt8.reshape(D, KP, 128, 2).transpose(2, 1, 0, 3))

    vrep = np.ascontiguousarray(
        np.broadcast_to(v_w[0].astype(np.float16)[None, :], (128, D))
    )
    return xin, wtl, vrep


def kernel(hidden, encoder_outputs, attn_w, attn_b, v_w, v_b):
    _install_ntff_hook()
    from concourse.bass_utils import run_bass_kernel_spmd

    hidden = np.asarray(hidden, dtype=np.float32)
    encoder_outputs = np.asarray(encoder_outputs, dtype=np.float32)
    attn_w = np.asarray(attn_w, dtype=np.float32)
    attn_b = np.asarray(attn_b, dtype=np.float32)
    v_w = np.asarray(v_w, dtype=np.float32)

    nc = _build()
    xin_full, wtl, vrep = _prep(hidden, encoder_outputs, attn_w, attn_b, v_w)

    in_maps = []
    for c in range(NCORES):
        b0 = c * BLOC
        in_maps.append(
            {
                "xin": np.ascontiguousarray(xin_full[b0 : b0 + BLOC]),
                "wt": wtl,
                "vrep": vrep,
            }
        )

    trace = bool(int(os.environ.get("KERNEL_TRACE", "0")))
    res = run_bass_kernel_spmd(
        nc, in_maps, core_ids=list(range(NCORES)), trace=trace
    )
    kernel.last_result = res

    outs = []
    for c in range(NCORES):
        ex = res.results[c]["out"]         # [128, BLOC, 16]
        sums = ex.astype(np.float64).sum(axis=(0, 2))           # [BLOC]
        o = ex / sums[None, :, None].astype(np.float32)
        # o[t, b, st*4+q] -> [b, s = st*512 + q*128 + t]
        o = o.reshape(128, BLOC, ST, Q).transpose(1, 2, 3, 0).reshape(BLOC, S)
        outs.append(o)
    full = np.concatenate(outs, axis=0)    # [B, S]
    full = full.transpose(1, 0).reshape(S, B, 1)
    return np.ascontiguousarray(full, dtype=np.float32)


kernel.last_result = None
